# revision 18
# baseline (speedup 1.0000x reference)
"""Trainium2 Bass kernel for nn_ByteSequenceEmbedder (fp8 DoubleRow version).

Model (per sequence, 8 sequences data-parallel over 8 NeuronCores):
  x  = tok_emb[tokens] + bpe*E[4] + word*E[3]                 [T=4096, 64]
  x  = relu(conv3(x, W0) + b0); 2x highway(512)               [T, 512]
  x  = relu(conv3(x, W1) + b1 + x); 2x highway(512)           [T, 512]
  x  = per-word segment max (ragged, sorted seg_ids, W=1024)  [W, 512]
  out= x @ Pw + Pb                                            [W, 512]

v2 strategy: the big GEMMs run as fp8e4m3 DoubleRow matmuls (2 K-tiles per
instruction at 0.5 cycles/row = 4x bf16 throughput). Precision is recovered
with cheap selective compensation (numerically validated, rel err ~1.3e-2):
 - activations y carried in bf16 (the "carrier"); matmul inputs are fp8
   casts yq produced by gpsimd cast-DMAs (DMA engines do the conversion)
 - gate path g = sigmoid(Wg@yq): raw fp8 (insensitive, validated)
 - h path / conv1: weight tensors split W = Whi + Wlo (both e4m3, shared
   scale) where configured; activation residual ylo = y - yq (fp8, shared
   scale) added as extra DoubleRow pair-groups where configured
 - conv1 residual folded into the center tap (identity += W1[1]) with its
   quantization error covered by the Wlo pair-group
 - conv0 and the output projection stay bf16 (small / precision-critical)
Scales (powers of 2, folded into weights / Act scale args):
  S0=64 (block-0 carrier), swg=128 (hw weights), sw1=4 (conv1), U1=sw1*S0.

Per-engine layout: Act does sigmoid/relu PSUM evictions; DVE does the
highway combine (3 tensor_tensor) + ylo residuals + transpose evictions +
segment-max tree; gpsimd (Pool) does the embedding/segmax gathers and all
bf16->fp8 cast-DMAs; PE does matmuls/transposes.
"""

import functools
import os
import sys

import numpy as np

for _p in ("/opt/trn_rl_repo", "/root/.axon_site/_ro/trn_rl_repo"):
    if os.path.isdir(_p) and _p not in sys.path:
        sys.path.append(_p)

import ml_dtypes  # noqa: E402

from contextlib import ExitStack  # noqa: E402

from concourse import bacc, bass, mybir, tile  # noqa: E402
from concourse import library_config  # noqa: E402
from concourse.bass_utils import run_bass_kernel_spmd  # noqa: E402

B, T, W = 8, 4096, 1024
DB, DW = 64, 512
NH = 2
VOCAB = 264
BPE_MARK, WORD_MARK = 4, 3
SC = 2048          # tokens per super-chunk (psum tile free size)
NSC = T // SC
NMM = 512          # bf16 matmul moving columns
NDR = 256          # DoubleRow out columns (rhs moving = 2*NDR)
MCH = DW // 128
KCH = DW // 128
KP = KCH // 2      # k-tile pairs
NCORES = 8
CVOCAB = 4 * VOCAB

S0 = 64.0          # block-0 carrier scale
SWG = 128.0        # highway weight scale
SW1 = 4.0          # conv1 weight scale
U1 = SW1 * S0      # block-1 carrier scale

# compensation config (validated in numpy: rel err ~1.53e-2)
WCOMP = {"h1l0", "h1l1", "c1"}
ACOMP = {"y0pp", "y1", "y1p"}

BF16 = mybir.dt.bfloat16
F32 = mybir.dt.float32
FP8 = mybir.dt.float8e4
I16 = mybir.dt.int16
AF = mybir.ActivationFunctionType
OP = mybir.AluOpType
PM = mybir.MatmulPerfMode

bf16_np = ml_dtypes.bfloat16
f8_np = ml_dtypes.float8_e4m3


def _col_bg(block, l):
    return 8 + block * 8 + l * 4


def _col_bh(block, l):
    return 24 + block * 8 + l * 4


def build_program(ntaps: int, dbg: bool = False) -> bass.Bass:
    nc = bacc.Bacc("TRN2", target_bir_lowering=False, debug=False)

    def din(name, shape, dtype):
        return nc.dram_tensor(name, list(shape), dtype, kind="ExternalInput")

    emb_d = din("emb_comb", (CVOCAB, 128), BF16)
    tokidx_d = din("tok_idx", (128, T // 16), I16)
    w0_d = din("w0", (DB, 3, DW), BF16)
    # fp8 weights laid out [128, ..., pair(2), DW]; slice [:, ..., j, :, m*128:+128]
    wg0_d = din("wg0", (128, NH, KP, 2, DW), FP8)
    wh0_d = din("wh0", (128, NH, KP, 2, DW), FP8)
    wg1_d = din("wg1", (128, NH, KP, 2, DW), FP8)
    wh1_d = din("wh1", (128, NH, KP, 2, DW), FP8)
    wh1lo_d = din("wh1lo", (128, NH, KP, 2, DW), FP8)
    c1hi_d = din("c1hi", (128, 3, KP, 2, DW), FP8)
    c1lo_d = din("c1lo", (128, 3, KP, 2, DW), FP8)
    projw_d = din("projw", (128, KCH, DW), BF16)
    projb_d = din("projb", (1, DW), BF16)
    bias_d = din("biases", (128, 40), F32)
    ident_d = din("ident", (128, 128), BF16)
    gidx_d = din("gidx", (128, 8 * ntaps * 8), I16)
    out_d = nc.dram_tensor("out", [W, DW], F32, kind="ExternalOutput")
    y1t_d = nc.dram_tensor("y1t", [T, DW], BF16, kind="Internal")
    dbg_d = None
    if dbg:
        dbg_d = nc.dram_tensor("dbg", [6, 128, KCH * T], BF16,
                               kind="ExternalOutput")

    with tile.TileContext(nc) as tc, ExitStack() as ctx:
        const = ctx.enter_context(tc.tile_pool(name="const", bufs=1))
        ps = ctx.enter_context(tc.tile_pool(name="psp", bufs=2, space="PSUM"))
        gp = ctx.enter_context(tc.tile_pool(name="gpool", bufs=4))
        hp = ctx.enter_context(tc.tile_pool(name="hpool", bufs=4))
        dp = ctx.enter_context(tc.tile_pool(name="dpool", bufs=2))
        ybfp = ctx.enter_context(tc.tile_pool(name="ybfp", bufs=1))
        yqp = ctx.enter_context(tc.tile_pool(name="yqp", bufs=2))
        ylop = ctx.enter_context(tc.tile_pool(name="ylop", bufs=1))
        tp = ctx.enter_context(tc.tile_pool(name="tpool", bufs=3))
        gat = ctx.enter_context(tc.tile_pool(name="gat", bufs=2))
        obp = ctx.enter_context(tc.tile_pool(name="obp", bufs=2))

        nc.gpsimd.load_library(library_config.mlp)

        def load(dram_t, shape, dtype, name):
            t = const.tile(shape, dtype, name=name)
            nc.sync.dma_start(out=t[:], in_=dram_t[:])
            return t

        tokidx_sb = load(tokidx_d, [128, T // 16], I16, "tokidx_sb")
        w0_sb = load(w0_d, [DB, 3, DW], BF16, "w0_sb")
        bias_sb = load(bias_d, [128, 40], F32, "bias_sb")

        # ---- embedding gather: xg[p, t] = emb_comb[cidx[t], p] ----
        xg = const.tile([128, T], BF16, name="xg")
        EC = T // 4
        for r in range(4):
            nc.gpsimd.dma_gather(
                out_ap=xg[:, r * EC:(r + 1) * EC].rearrange(
                    "p (c n) -> p c n", c=1),
                in_ap=emb_d[:],
                idxs_ap=tokidx_sb[:, r * (EC // 16):(r + 1) * (EC // 16)],
                num_idxs=EC,
                num_idxs_reg=EC,
                elem_size=128,
                transpose=True,
                single_packet=False,
            )

        wg0_sb = load(wg0_d, [128, NH, KP, 2, DW], FP8, "wg0_sb")
        wh0_sb = load(wh0_d, [128, NH, KP, 2, DW], FP8, "wh0_sb")
        wg1_sb = load(wg1_d, [128, NH, KP, 2, DW], FP8, "wg1_sb")
        wh1_sb = load(wh1_d, [128, NH, KP, 2, DW], FP8, "wh1_sb")
        wh1lo_sb = load(wh1lo_d, [128, NH, KP, 2, DW], FP8, "wh1lo_sb")
        c1hi_sb = load(c1hi_d, [128, 3, KP, 2, DW], FP8, "c1hi_sb")
        c1lo_sb = load(c1lo_d, [128, 3, KP, 2, DW], FP8, "c1lo_sb")
        projw_sb = load(projw_d, [128, KCH, DW], BF16, "projw_sb")
        projb_sb = load(projb_d, [1, DW], BF16, "projb_sb")
        gidx_sb = load(gidx_d, [128, 8 * ntaps * 8], I16, "gidx_sb")
        ident_sb = load(ident_d, [128, 128], BF16, "ident_sb")
        ones_sb = const.tile([1, 128], BF16, name="ones_sb")
        nc.vector.memset(ones_sb[:], 1.0)

        # ---- conv0 (bf16, weights carry S0), relu -> ybf0 [128, KCH, T] ----
        ybf0 = ybfp.tile([128, KCH, T], BF16, tag="ybf", name="ybf0")
        yq0 = yqp.tile([128, KCH, T], FP8, tag="yq", name="yq0")
        for sc in range(NSC):
            base = sc * SC
            for m in range(MCH):
                pc = ps.tile([128, SC], F32, tag="ps", name="pc")
                order = [1, 0, 2]
                for oi, k in enumerate(order):
                    for n in range(SC // NMM):
                        t0 = base + n * NMM
                        col0, col1 = n * NMM, (n + 1) * NMM
                        lo = t0 + (k - 1)
                        ln = NMM
                        o0, o1 = col0, col1
                        if lo < 0:
                            lo, ln, o0 = 0, NMM - 1, col0 + 1
                        elif lo + ln > T:
                            ln, o1 = T - lo, col1 - 1
                        nc.tensor.matmul(
                            out=pc[:, o0:o1],
                            lhsT=w0_sb[:, k, m * 128:(m + 1) * 128],
                            rhs=xg[0:DB, lo:lo + ln],
                            start=(oi == 0),
                            stop=(oi == 2),
                        )
                nc.scalar.activation(
                    out=ybf0[:, m, base:base + SC], in_=pc[:],
                    func=AF.Relu, bias=bias_sb[:, m:m + 1],
                )
            nc.gpsimd.dma_start(out=yq0[:, :, base:base + SC],
                                in_=ybf0[:, :, base:base + SC])

        def cast_fp8(dst, src, sc):
            """gpsimd cast-DMA of one superchunk [128, KCH, SC]."""
            base = sc * SC
            nc.gpsimd.dma_start(out=dst[:, :, base:base + SC],
                                in_=src[:, :, base:base + SC])

        def dump(i, t):
            if dbg_d is not None:
                nc.sync.dma_start(out=dbg_d[i], in_=t[:])

        dump(0, ybf0)

        # ---- DoubleRow helpers ----
        def dr_gemm(pt, base, groups):
            """Accumulate DoubleRow pair-groups into psum tile pt [128, SC].

            groups: list of (lhsT_fn(j, m), rhs_t, dt) where rhs_t is an
            fp8 [128, KCH, T] tensor read at token offset dt (clipped at
            the sequence edge; clipped columns simply miss that tap, which
            matches SAME padding). First group's first pair must cover all
            columns -> callers put an unshifted (dt=0) group first."""
            m = dr_gemm.m
            ng = len(groups)
            # n outermost: each 256-col chunk completes its full accumulation
            # group (start..stop) before the next chunk's start re-marks the
            # shared 2KB PSUM bank's zero region.
            for n in range(SC // NDR):
                for gi, (lf, rhs_t, dt) in enumerate(groups):
                    for j in range(KP):
                        t0 = base + n * NDR
                        col0, col1 = n * NDR, (n + 1) * NDR
                        lo = t0 + dt
                        ln = NDR
                        o0, o1 = col0, col1
                        if lo < 0:
                            lo, ln, o0 = 0, NDR - 1, col0 + 1
                        elif lo + ln > T:
                            ln, o1 = T - lo, col1 - 1
                        nc.tensor.matmul(
                            out=pt[:, o0:o1],
                            lhsT=lf(j, m),
                            rhs=rhs_t[:, 2 * j:2 * j + 2, lo:lo + ln],
                            start=(gi == 0 and j == 0),
                            stop=(gi == ng - 1 and j == KP - 1),
                            perf_mode=PM.DoubleRow,
                        )

        def alloc_pieces(comp, nm):
            yq = yqp.tile([128, KCH, T], FP8, tag="yq", name=f"yq_{nm}")
            ylo = None
            if comp:
                ylo = ylop.tile([128, KCH, T], FP8, tag="ylo", name=f"ylo_{nm}")
            return yq, ylo

        def emit_pieces_sc(ybf, pieces, sc):
            """per-superchunk cast (+ ylo residual) right after the combine,
            so the next layer's matmuls for this sc unblock early."""
            if pieces is None:
                return
            yq_n, ylo_n = pieces
            base = sc * SC
            cast_fp8(yq_n, ybf, sc)
            if ylo_n is not None:
                nc.vector.tensor_tensor(
                    out=ylo_n[:, :, base:base + SC],
                    in0=ybf[:, :, base:base + SC],
                    in1=yq_n[:, :, base:base + SC],
                    op=OP.subtract)

        # ---- highway phase ----
        def hw_phase(blk, l, name, ybf, yq, ylo, wg_sb, wh_sb, whlo_sb,
                     sig_scale, pieces_next=None, post_sc=None):
            colg, colh = _col_bg(blk, l), _col_bh(blk, l)
            for sc in range(NSC):
                base = sc * SC
                g_tiles = []
                for m in range(MCH):
                    dr_gemm.m = m
                    pg = ps.tile([128, SC], F32, tag="ps", name="pg")
                    dr_gemm(pg, base, [
                        (lambda j, m: wg_sb[:, l, j, :, m * 128:(m + 1) * 128],
                         yq, 0),
                    ])
                    g = gp.tile([128, SC], BF16, tag="g", name="g")
                    nc.scalar.activation(
                        out=g[:], in_=pg[:], func=AF.Sigmoid,
                        bias=bias_sb[:, colg + m:colg + m + 1],
                        scale=sig_scale,
                    )
                    g_tiles.append(g)
                h_tiles = []
                for m in range(MCH):
                    dr_gemm.m = m
                    ph = ps.tile([128, SC], F32, tag="ps", name="ph")
                    groups = [
                        (lambda j, m: wh_sb[:, l, j, :, m * 128:(m + 1) * 128],
                         yq, 0),
                    ]
                    if whlo_sb is not None:
                        groups.append(
                            (lambda j, m: whlo_sb[:, l, j, :,
                                                  m * 128:(m + 1) * 128],
                             yq, 0))
                    if ylo is not None:
                        groups.append(
                            (lambda j, m: wh_sb[:, l, j, :,
                                                m * 128:(m + 1) * 128],
                             ylo, 0))
                    dr_gemm(ph, base, groups)
                    h = hp.tile([128, SC], BF16, tag="h", name="h")
                    nc.scalar.activation(
                        out=h[:], in_=ph[:], func=AF.Relu,
                        bias=bias_sb[:, colh + m:colh + m + 1],
                        scale=1.0 / SWG,
                    )
                    h_tiles.append(h)
                for m in range(MCH):
                    yap = ybf[:, m, base:base + SC]
                    d = dp.tile([128, SC], BF16, tag="d", name="d")
                    nc.vector.tensor_tensor(
                        out=d[:], in0=h_tiles[m][:], in1=yap, op=OP.subtract)
                    nc.vector.tensor_tensor(
                        out=d[:], in0=g_tiles[m][:], in1=d[:], op=OP.mult)
                    nc.vector.tensor_tensor(
                        out=yap, in0=yap, in1=d[:], op=OP.add)
                emit_pieces_sc(ybf, pieces_next, sc)
                if post_sc is not None:
                    post_sc(sc)

        # block 0
        p_y0p = alloc_pieces("y0p" in ACOMP, "y0p")
        hw_phase(0, 0, "h0l0", ybf0, yq0, None, wg0_sb, wh0_sb,
                 None, 1.0 / (SWG * S0), pieces_next=p_y0p)
        dump(1, ybf0)
        p_y0pp = alloc_pieces("y0pp" in ACOMP, "y0pp")
        hw_phase(0, 1, "h0l1", ybf0, p_y0p[0], p_y0p[1], wg0_sb, wh0_sb,
                 None, 1.0 / (SWG * S0), pieces_next=p_y0pp)
        dump(2, ybf0)
        yq0pp, ylo0pp = p_y0pp

        # ---- conv1 (fp8 DoubleRow; residual identity in center tap) ----
        ybf1 = ybfp.tile([128, KCH, T], BF16, tag="ybf", name="ybf1")
        p_y1 = alloc_pieces("y1" in ACOMP, "y1")
        for sc in range(NSC):
            base = sc * SC
            for m in range(MCH):
                dr_gemm.m = m
                pc = ps.tile([128, SC], F32, tag="ps", name="pc1")
                groups = [
                    (lambda j, m: c1hi_sb[:, 1, j, :, m * 128:(m + 1) * 128],
                     yq0pp, 0),
                    (lambda j, m: c1hi_sb[:, 0, j, :, m * 128:(m + 1) * 128],
                     yq0pp, -1),
                    (lambda j, m: c1hi_sb[:, 2, j, :, m * 128:(m + 1) * 128],
                     yq0pp, 1),
                ]
                for k, dt in ((1, 0), (0, -1), (2, 1)):
                    groups.append(
                        (lambda j, m, k=k: c1lo_sb[:, k, j, :,
                                                   m * 128:(m + 1) * 128],
                         yq0pp, dt))
                if ylo0pp is not None:
                    for k, dt in ((1, 0), (0, -1), (2, 1)):
                        groups.append(
                            (lambda j, m, k=k: c1hi_sb[:, k, j, :,
                                                       m * 128:(m + 1) * 128],
                             ylo0pp, dt))
                dr_gemm(pc, base, groups)
                nc.scalar.activation(
                    out=ybf1[:, m, base:base + SC], in_=pc[:],
                    func=AF.Relu, bias=bias_sb[:, 4 + m:5 + m],
                )
            emit_pieces_sc(ybf1, p_y1, sc)

        dump(3, ybf1)
        p_y1p = alloc_pieces("y1p" in ACOMP, "y1p")
        hw_phase(1, 0, "h1l0", ybf1, p_y1[0], p_y1[1], wg1_sb, wh1_sb,
                 wh1lo_sb if "h1l0" in WCOMP else None, 1.0 / (SWG * U1),
                 pieces_next=p_y1p)
        dump(4, ybf1)

        # ---- tail (emitted per-sc inside h1l1): transpose ybf1 ->
        # token-major y1t in DRAM; then per word-chunk gather + max tree +
        # proj as soon as the needed y1t rows exist ----
        a2_all = const.tile([128, KCH, W], BF16, name="a2_all")

        def seg_chunk(wc):
            tap = gat.tile([128, KCH, ntaps * 128], BF16, tag="tap", name="tap")
            rmax = min(((ntaps * 128 * (wc + 1) + 127) // 128) * 128, T)
            nc.gpsimd.dma_gather(
                out_ap=tap[:],
                in_ap=y1t_d[0:rmax, :],
                idxs_ap=gidx_sb[:, wc * ntaps * 8:(wc + 1) * ntaps * 8],
                num_idxs=ntaps * 128,
                num_idxs_reg=ntaps * 128,
                elem_size=DW,
                transpose=True,
                single_packet=False,
            )
            a2s = a2_all[:, :, wc * 128:(wc + 1) * 128]
            nc.vector.tensor_tensor(
                out=a2s, in0=tap[:, :, 0:128], in1=tap[:, :, 128:256], op=OP.max)
            for j in range(2, ntaps):
                nc.vector.tensor_tensor(
                    out=a2s, in0=a2s, in1=tap[:, :, j * 128:(j + 1) * 128],
                    op=OP.max)
            # proj (bf16; 1/U1 folded into projw) + bias, fp32 out
            po = ps.tile([128, DW], F32, tag="ps", name="po")
            for k in range(KCH):
                nc.tensor.matmul(
                    out=po[:],
                    lhsT=a2_all[:, k, wc * 128:(wc + 1) * 128],
                    rhs=projw_sb[:, k, :],
                    start=(k == 0),
                    stop=False,
                )
            nc.tensor.matmul(
                out=po[:], lhsT=ones_sb[:, 0:128], rhs=projb_sb[:],
                start=False, stop=True,
            )
            ob = obp.tile([128, DW], F32, tag="ob", name="ob")
            nc.scalar.activation(out=ob[:], in_=po[:], func=AF.Copy)
            nc.sync.dma_start(out=out_d[wc * 128:(wc + 1) * 128, :], in_=ob[:])

        def tail_sc(sc):
            for i in range(SC // 128):
                pt = ps.tile([128, 512], BF16, tag="ps", name="pt")
                for m in range(MCH):
                    nc.tensor.transpose(
                        out=pt[:, m * 128:(m + 1) * 128],
                        in_=ybf1[:, m, sc * SC + i * 128:sc * SC + (i + 1) * 128],
                        identity=ident_sb[:],
                    )
                st = tp.tile([128, 512], BF16, tag="y1t", name="st")
                nc.vector.tensor_copy(out=st[:], in_=pt[:])
                row0 = (sc * (SC // 128) + i) * 128
                nc.sync.dma_start(out=y1t_d[row0:row0 + 128, :], in_=st[:])
            hi = (sc + 1) * SC
            lo_r = sc * SC
            for wc in range(8):
                rmax = min(((ntaps * 128 * (wc + 1) + 127) // 128) * 128, T)
                ready_now = rmax <= hi
                ready_before = rmax <= lo_r
                if ready_now and not ready_before:
                    seg_chunk(wc)
                elif sc == NSC - 1 and not ready_now:
                    seg_chunk(wc)

        hw_phase(1, 1, "h1l1", ybf1, p_y1p[0], p_y1p[1], wg1_sb, wh1_sb,
                 wh1lo_sb if "h1l1" in WCOMP else None, 1.0 / (SWG * U1),
                 post_sc=tail_sc)
        dump(5, ybf1)

    nc.compile()
    return nc


@functools.lru_cache(maxsize=2)
def _program(ntaps: int) -> bass.Bass:
    return build_program(ntaps)


def _pack_idx(lin: np.ndarray) -> np.ndarray:
    """SWDGE idx layout: [128, N/16] int16, value n at [p, s] with
    n = s*16 + p%16, replicated across the eight 16-partition groups."""
    n = len(lin)
    assert n % 16 == 0
    arr = np.asarray(lin, dtype=np.int16).reshape(n // 16, 16).T
    return np.tile(arr, (8, 1)).copy()


def _q8(x):
    return np.clip(x, -240.0, 240.0).astype(f8_np)


def _wpack(w, sw, lo=False):
    """[512, 512] weight -> hi or lo fp8 piece, layout [128, KP, 2, 512]."""
    ws = w * sw
    hi = _q8(ws).astype(np.float32)
    src = (ws - hi) if lo else ws
    q = _q8(src)                                   # [512 in, 512 out]
    # in-feature f = (2j+i)*128 + p  ->  [p, j, i, out]
    return np.ascontiguousarray(
        np.transpose(q.reshape(KP, 2, 128, DW), (2, 0, 1, 3)))


def prepare(inputs):
    f32 = np.float32
    bt = np.asarray(inputs["byte_tokens"]).astype(np.int64)
    bpe = np.asarray(inputs["bpe_mask"]).astype(np.int64)
    wrd = np.asarray(inputs["word_mask"]).astype(np.int64)
    seg = np.asarray(inputs["seg_ids"]).astype(np.int64)
    emb = np.asarray(inputs["tok_emb"], dtype=f32)
    conv0_w = np.asarray(inputs["conv0_w"], dtype=f32)
    conv0_b = np.asarray(inputs["conv0_b"], dtype=f32)
    conv1_w = np.asarray(inputs["conv1_w"], dtype=f32)
    conv1_b = np.asarray(inputs["conv1_b"], dtype=f32)
    hw_w = {
        (0, "g"): np.asarray(inputs["hw0_wg"], dtype=f32),
        (0, "h"): np.asarray(inputs["hw0_wh"], dtype=f32),
        (1, "g"): np.asarray(inputs["hw1_wg"], dtype=f32),
        (1, "h"): np.asarray(inputs["hw1_wh"], dtype=f32),
    }
    hw_b = {
        (0, "g"): np.asarray(inputs["hw0_bg"], dtype=f32),
        (0, "h"): np.asarray(inputs["hw0_bh"], dtype=f32),
        (1, "g"): np.asarray(inputs["hw1_bg"], dtype=f32),
        (1, "h"): np.asarray(inputs["hw1_bh"], dtype=f32),
    }
    proj_w = np.asarray(inputs["proj_w"], dtype=f32)
    proj_b = np.asarray(inputs["proj_b"], dtype=f32)

    def as_bf16(x):
        return np.ascontiguousarray(x.astype(bf16_np))

    # combined embedding table (bf16)
    embc = np.zeros((CVOCAB, 128), f32)
    for bm in range(2):
        for wm in range(2):
            r0 = VOCAB * (bm + 2 * wm)
            embc[r0:r0 + VOCAB, :DB] = (
                emb + bm * emb[BPE_MARK] + wm * emb[WORD_MARK])

    def hwpack(block, gh, lo=False):
        # [NH, 512, 512] -> [128, NH, KP, 2, 512] fp8 at scale SWG
        return np.ascontiguousarray(np.stack(
            [_wpack(hw_w[(block, gh)][l], SWG, lo) for l in range(NH)],
            axis=1))

    c1 = conv1_w.copy()
    c1[1] += np.eye(DW, dtype=f32)

    def c1pack(lo=False):
        return np.ascontiguousarray(np.stack(
            [_wpack(c1[k], SW1, lo) for k in range(3)], axis=1))

    shared = {
        "emb_comb": as_bf16(embc),
        "w0": as_bf16(np.transpose(conv0_w * S0, (1, 0, 2))),    # [64,3,512]
        "wg0": hwpack(0, "g"),
        "wh0": hwpack(0, "h"),
        "wg1": hwpack(1, "g"),
        "wh1": hwpack(1, "h"),
        "wh1lo": hwpack(1, "h", lo=True),
        "c1hi": c1pack(),
        "c1lo": c1pack(lo=True),
        "projw": as_bf16(np.transpose(
            (proj_w / U1).reshape(KCH, 128, DW), (1, 0, 2))),
        "projb": as_bf16(proj_b.reshape(1, DW)),
        "ident": np.eye(128, dtype=bf16_np),
    }

    # biases (columns): conv0*S0, conv1*U1, sigmoid raw, relu-h*S_carrier
    bias_h = np.zeros((128, 40), f32)
    bias_h[:, 0:4] = (conv0_b * S0).reshape(KCH, 128).T
    bias_h[:, 4:8] = (conv1_b * U1).reshape(KCH, 128).T
    for blk, Sc in ((0, S0), (1, U1)):
        for l in range(NH):
            bias_h[:, _col_bg(blk, l):_col_bg(blk, l) + 4] = \
                hw_b[(blk, "g")][l].reshape(KCH, 128).T
            bias_h[:, _col_bh(blk, l):_col_bh(blk, l) + 4] = \
                (hw_b[(blk, "h")][l] * Sc).reshape(KCH, 128).T
    shared["biases"] = np.ascontiguousarray(bias_h)

    # per-core seg prep
    counts = np.zeros((B, W), np.int64)
    for b in range(B):
        counts[b] = np.bincount(seg[b], minlength=W)[:W]
    assert (counts >= 1).all(), "empty segments unsupported"
    ntaps = max(int(counts.max()), 2)
    starts = np.zeros((B, W), np.int64)
    starts[:, 1:] = np.cumsum(counts, axis=1)[:, :-1]
    ends = starts + counts - 1

    in_maps = []
    for b in range(B):
        cidx = bt[b] + VOCAB * (bpe[b] + 2 * wrd[b])
        gl = np.empty(8 * ntaps * 128, np.int64)
        for wc in range(8):
            nvec = np.arange(ntaps * 128)
            wv = wc * 128 + (nvec % 128)
            jv = nvec // 128
            gl[wc * ntaps * 128:(wc + 1) * ntaps * 128] = np.minimum(
                starts[b, wv] + jv, ends[b, wv]
            )
        m = dict(shared)
        m["tok_idx"] = _pack_idx(cidx)
        m["gidx"] = np.concatenate(
            [_pack_idx(gl[wc * ntaps * 128:(wc + 1) * ntaps * 128])
             for wc in range(8)], axis=1
        ).copy()
        in_maps.append(m)
    return ntaps, in_maps


def _run(inputs, trace=False, **kwargs):
    ntaps, in_maps = prepare(inputs)
    nc = _program(ntaps)
    res = run_bass_kernel_spmd(
        nc, in_maps, core_ids=list(range(NCORES)), trace=trace, **kwargs
    )
    out = np.stack([res.results[b]["out"] for b in range(B)], axis=0)
    return out.astype(np.float32), res


def kernel(**inputs) -> np.ndarray:
    out, _ = _run(inputs, trace=False)
    return out


def run_traced(inputs, **kwargs):
    return _run(inputs, trace=True, **kwargs)


# revision 22
# speedup vs baseline: 1.0720x; 1.0720x over previous
"""Trainium2 Bass kernel for nn_ByteSequenceEmbedder (fp8 DoubleRow version).

Model (per sequence, 8 sequences data-parallel over 8 NeuronCores):
  x  = tok_emb[tokens] + bpe*E[4] + word*E[3]                 [T=4096, 64]
  x  = relu(conv3(x, W0) + b0); 2x highway(512)               [T, 512]
  x  = relu(conv3(x, W1) + b1 + x); 2x highway(512)           [T, 512]
  x  = per-word segment max (ragged, sorted seg_ids, W=1024)  [W, 512]
  out= x @ Pw + Pb                                            [W, 512]

v2 strategy: the big GEMMs run as fp8e4m3 DoubleRow matmuls (2 K-tiles per
instruction at 0.5 cycles/row = 4x bf16 throughput). Precision is recovered
with cheap selective compensation (numerically validated, rel err ~1.3e-2):
 - activations y carried in bf16 (the "carrier"); matmul inputs are fp8
   casts yq produced by gpsimd cast-DMAs (DMA engines do the conversion)
 - gate path g = sigmoid(Wg@yq): raw fp8 (insensitive, validated)
 - h path / conv1: weight tensors split W = Whi + Wlo (both e4m3, shared
   scale) where configured; activation residual ylo = y - yq (fp8, shared
   scale) added as extra DoubleRow pair-groups where configured
 - conv1 residual folded into the center tap (identity += W1[1]) with its
   quantization error covered by the Wlo pair-group
 - conv0 and the output projection stay bf16 (small / precision-critical)
Scales (powers of 2, folded into weights / Act scale args):
  S0=64 (block-0 carrier), swg=128 (hw weights), sw1=4 (conv1), U1=sw1*S0.

Per-engine layout: Act does sigmoid/relu PSUM evictions; DVE does the
highway combine (3 tensor_tensor) + ylo residuals + transpose evictions +
segment-max tree; gpsimd (Pool) does the embedding/segmax gathers and all
bf16->fp8 cast-DMAs; PE does matmuls/transposes.
"""

import functools
import os
import sys

import numpy as np

for _p in ("/opt/trn_rl_repo", "/root/.axon_site/_ro/trn_rl_repo"):
    if os.path.isdir(_p) and _p not in sys.path:
        sys.path.append(_p)

import ml_dtypes  # noqa: E402

from contextlib import ExitStack  # noqa: E402

from concourse import bacc, bass, mybir, tile  # noqa: E402
from concourse import library_config  # noqa: E402
from concourse.bass_utils import run_bass_kernel_spmd  # noqa: E402

B, T, W = 8, 4096, 1024
DB, DW = 64, 512
NH = 2
VOCAB = 264
BPE_MARK, WORD_MARK = 4, 3
SC = 1024          # tokens per super-chunk (psum tile free size)
NSC = T // SC
NMM = 512          # bf16 matmul moving columns
NDR = 256          # DoubleRow out columns (rhs moving = 2*NDR)
MCH = DW // 128
KCH = DW // 128
KP = KCH // 2      # k-tile pairs
NCORES = 8
CVOCAB = 4 * VOCAB

S0 = 64.0          # block-0 carrier scale
SWG = 128.0        # highway weight scale
SW1 = 4.0          # conv1 weight scale
U1 = SW1 * S0      # block-1 carrier scale

# compensation config (validated in numpy: rel err ~1.53e-2)
WCOMP = {"h1l0", "h1l1", "c1"}
ACOMP = {"y0pp", "y1", "y1p"}

BF16 = mybir.dt.bfloat16
F32 = mybir.dt.float32
FP8 = mybir.dt.float8e4
I16 = mybir.dt.int16
AF = mybir.ActivationFunctionType
OP = mybir.AluOpType
PM = mybir.MatmulPerfMode

bf16_np = ml_dtypes.bfloat16
f8_np = ml_dtypes.float8_e4m3


def _col_bg(block, l):
    return 8 + block * 8 + l * 4


def _col_bh(block, l):
    return 24 + block * 8 + l * 4


def build_program(ntaps: int, dbg: bool = False) -> bass.Bass:
    nc = bacc.Bacc("TRN2", target_bir_lowering=False, debug=False)

    def din(name, shape, dtype):
        return nc.dram_tensor(name, list(shape), dtype, kind="ExternalInput")

    emb_d = din("emb_comb", (CVOCAB, 128), BF16)
    tokidx_d = din("tok_idx", (128, T // 16), I16)
    w0_d = din("w0", (DB, 3, DW), BF16)
    # fp8 weights laid out [128, ..., pair(2), DW]; slice [:, ..., j, :, m*128:+128]
    wg0_d = din("wg0", (128, NH, KP, 2, DW), FP8)
    wh0_d = din("wh0", (128, NH, KP, 2, DW), FP8)
    wg1_d = din("wg1", (128, NH, KP, 2, DW), FP8)
    wh1_d = din("wh1", (128, NH, KP, 2, DW), FP8)
    wh1lo_d = din("wh1lo", (128, NH, KP, 2, DW), FP8)
    c1hi_d = din("c1hi", (128, 3, KP, 2, DW), FP8)
    c1lo_d = din("c1lo", (128, 3, KP, 2, DW), FP8)
    projw_d = din("projw", (128, KCH, DW), BF16)
    projb_d = din("projb", (1, DW), BF16)
    bias_d = din("biases", (128, 40), F32)
    ident_d = din("ident", (128, 128), BF16)
    gidx_d = din("gidx", (128, 8 * ntaps * 8), I16)
    out_d = nc.dram_tensor("out", [W, DW], F32, kind="ExternalOutput")
    y1t_d = nc.dram_tensor("y1t", [T, DW], BF16, kind="Internal")
    dbg_d = None
    if dbg:
        dbg_d = nc.dram_tensor("dbg", [6, 128, KCH * T], BF16,
                               kind="ExternalOutput")

    with tile.TileContext(nc) as tc, ExitStack() as ctx:
        const = ctx.enter_context(tc.tile_pool(name="const", bufs=1))
        ps = ctx.enter_context(tc.tile_pool(name="psp", bufs=3, space="PSUM"))
        tps = ctx.enter_context(tc.tile_pool(name="tailps", bufs=2,
                                             space="PSUM"))
        gp = ctx.enter_context(tc.tile_pool(name="gpool", bufs=4))
        hp = ctx.enter_context(tc.tile_pool(name="hpool", bufs=4))
        dp = ctx.enter_context(tc.tile_pool(name="dpool", bufs=2))
        ybfp = ctx.enter_context(tc.tile_pool(name="ybfp", bufs=1))
        yqp = ctx.enter_context(tc.tile_pool(name="yqp", bufs=2))
        ylop = ctx.enter_context(tc.tile_pool(name="ylop", bufs=1))
        tp = ctx.enter_context(tc.tile_pool(name="tpool", bufs=3))
        gat = ctx.enter_context(tc.tile_pool(name="gat", bufs=2))
        obp = ctx.enter_context(tc.tile_pool(name="obp", bufs=2))

        nc.gpsimd.load_library(library_config.mlp)

        def load(dram_t, shape, dtype, name):
            t = const.tile(shape, dtype, name=name)
            nc.sync.dma_start(out=t[:], in_=dram_t[:])
            return t

        tokidx_sb = load(tokidx_d, [128, T // 16], I16, "tokidx_sb")
        w0_sb = load(w0_d, [DB, 3, DW], BF16, "w0_sb")
        bias_sb = load(bias_d, [128, 40], F32, "bias_sb")

        # ---- embedding gather: xg[p, t] = emb_comb[cidx[t], p] ----
        xg = const.tile([128, T], BF16, name="xg")
        EC = T // 4
        for r in range(4):
            nc.gpsimd.dma_gather(
                out_ap=xg[:, r * EC:(r + 1) * EC].rearrange(
                    "p (c n) -> p c n", c=1),
                in_ap=emb_d[:],
                idxs_ap=tokidx_sb[:, r * (EC // 16):(r + 1) * (EC // 16)],
                num_idxs=EC,
                num_idxs_reg=EC,
                elem_size=128,
                transpose=True,
                single_packet=False,
            )

        wg0_sb = load(wg0_d, [128, NH, KP, 2, DW], FP8, "wg0_sb")
        wh0_sb = load(wh0_d, [128, NH, KP, 2, DW], FP8, "wh0_sb")
        wg1_sb = load(wg1_d, [128, NH, KP, 2, DW], FP8, "wg1_sb")
        wh1_sb = load(wh1_d, [128, NH, KP, 2, DW], FP8, "wh1_sb")
        wh1lo_sb = load(wh1lo_d, [128, NH, KP, 2, DW], FP8, "wh1lo_sb")
        c1hi_sb = load(c1hi_d, [128, 3, KP, 2, DW], FP8, "c1hi_sb")
        c1lo_sb = load(c1lo_d, [128, 3, KP, 2, DW], FP8, "c1lo_sb")
        projw_sb = load(projw_d, [128, KCH, DW], BF16, "projw_sb")
        projb_sb = load(projb_d, [1, DW], BF16, "projb_sb")
        gidx_sb = load(gidx_d, [128, 8 * ntaps * 8], I16, "gidx_sb")
        ident_sb = load(ident_d, [128, 128], BF16, "ident_sb")
        ones_sb = const.tile([1, 128], BF16, name="ones_sb")
        nc.vector.memset(ones_sb[:], 1.0)

        # ---- conv0 (bf16, weights carry S0), relu -> ybf0 [128, KCH, T] ----
        ybf0 = ybfp.tile([128, KCH, T], BF16, tag="ybf", name="ybf0")
        yq0 = yqp.tile([128, KCH, T], FP8, tag="yq", name="yq0")
        for sc in range(NSC):
            base = sc * SC
            for m in range(MCH):
                pc = ps.tile([128, SC], F32, tag="ps", name="pc")
                order = [1, 0, 2]
                for oi, k in enumerate(order):
                    for n in range(SC // NMM):
                        t0 = base + n * NMM
                        col0, col1 = n * NMM, (n + 1) * NMM
                        lo = t0 + (k - 1)
                        ln = NMM
                        o0, o1 = col0, col1
                        if lo < 0:
                            lo, ln, o0 = 0, NMM - 1, col0 + 1
                        elif lo + ln > T:
                            ln, o1 = T - lo, col1 - 1
                        nc.tensor.matmul(
                            out=pc[:, o0:o1],
                            lhsT=w0_sb[:, k, m * 128:(m + 1) * 128],
                            rhs=xg[0:DB, lo:lo + ln],
                            start=(oi == 0),
                            stop=(oi == 2),
                        )
                nc.scalar.activation(
                    out=ybf0[:, m, base:base + SC], in_=pc[:],
                    func=AF.Relu, bias=bias_sb[:, m:m + 1],
                )
            nc.gpsimd.dma_start(out=yq0[:, :, base:base + SC],
                                in_=ybf0[:, :, base:base + SC])

        def cast_fp8(dst, src, sc):
            """gpsimd cast-DMA of one superchunk [128, KCH, SC]."""
            base = sc * SC
            nc.gpsimd.dma_start(out=dst[:, :, base:base + SC],
                                in_=src[:, :, base:base + SC])

        def dump(i, t):
            if dbg_d is not None:
                nc.sync.dma_start(out=dbg_d[i], in_=t[:])

        dump(0, ybf0)

        # ---- DoubleRow helpers ----
        def dr_gemm(pt, base, groups):
            """Accumulate DoubleRow pair-groups into psum tile pt [128, SC].

            groups: list of (lhsT_fn(j, m), rhs_t, dt) where rhs_t is an
            fp8 [128, KCH, T] tensor read at token offset dt (clipped at
            the sequence edge; clipped columns simply miss that tap, which
            matches SAME padding). First group's first pair must cover all
            columns -> callers put an unshifted (dt=0) group first."""
            m = dr_gemm.m
            ng = len(groups)
            # n outermost: each 256-col chunk completes its full accumulation
            # group (start..stop) before the next chunk's start re-marks the
            # shared 2KB PSUM bank's zero region.
            for n in range(SC // NDR):
                for gi, (lf, rhs_t, dt) in enumerate(groups):
                    for j in range(KP):
                        t0 = base + n * NDR
                        col0, col1 = n * NDR, (n + 1) * NDR
                        lo = t0 + dt
                        ln = NDR
                        o0, o1 = col0, col1
                        if lo < 0:
                            lo, ln, o0 = 0, NDR - 1, col0 + 1
                        elif lo + ln > T:
                            ln, o1 = T - lo, col1 - 1
                        nc.tensor.matmul(
                            out=pt[:, o0:o1],
                            lhsT=lf(j, m),
                            rhs=rhs_t[:, 2 * j:2 * j + 2, lo:lo + ln],
                            start=(gi == 0 and j == 0),
                            stop=(gi == ng - 1 and j == KP - 1),
                            perf_mode=PM.DoubleRow,
                        )

        def alloc_pieces(comp, nm):
            yq = yqp.tile([128, KCH, T], FP8, tag="yq", name=f"yq_{nm}")
            ylo = None
            if comp:
                ylo = ylop.tile([128, KCH, T], FP8, tag="ylo", name=f"ylo_{nm}")
            return yq, ylo

        def emit_pieces_sc(ybf, pieces, sc):
            """per-superchunk cast (+ ylo residual) right after the combine,
            so the next layer's matmuls for this sc unblock early."""
            if pieces is None:
                return
            yq_n, ylo_n = pieces
            base = sc * SC
            cast_fp8(yq_n, ybf, sc)
            if ylo_n is not None:
                nc.vector.tensor_tensor(
                    out=ylo_n[:, :, base:base + SC],
                    in0=ybf[:, :, base:base + SC],
                    in1=yq_n[:, :, base:base + SC],
                    op=OP.subtract)

        # ---- highway phase ----
        def hw_phase(blk, l, name, ybf, yq, ylo, wg_sb, wh_sb, whlo_sb,
                     sig_scale, pieces_next=None, post_sc=None):
            colg, colh = _col_bg(blk, l), _col_bh(blk, l)
            for sc in range(NSC):
                base = sc * SC
                g_tiles = []
                for m in range(MCH):
                    dr_gemm.m = m
                    pg = ps.tile([128, SC], F32, tag="ps", name="pg")
                    dr_gemm(pg, base, [
                        (lambda j, m: wg_sb[:, l, j, :, m * 128:(m + 1) * 128],
                         yq, 0),
                    ])
                    g = gp.tile([128, SC], BF16, tag="g", name="g")
                    nc.scalar.activation(
                        out=g[:], in_=pg[:], func=AF.Sigmoid,
                        bias=bias_sb[:, colg + m:colg + m + 1],
                        scale=sig_scale,
                    )
                    g_tiles.append(g)
                h_tiles = []
                for m in range(MCH):
                    dr_gemm.m = m
                    ph = ps.tile([128, SC], F32, tag="ps", name="ph")
                    groups = [
                        (lambda j, m: wh_sb[:, l, j, :, m * 128:(m + 1) * 128],
                         yq, 0),
                    ]
                    if whlo_sb is not None:
                        groups.append(
                            (lambda j, m: whlo_sb[:, l, j, :,
                                                  m * 128:(m + 1) * 128],
                             yq, 0))
                    if ylo is not None:
                        groups.append(
                            (lambda j, m: wh_sb[:, l, j, :,
                                                m * 128:(m + 1) * 128],
                             ylo, 0))
                    dr_gemm(ph, base, groups)
                    h = hp.tile([128, SC], BF16, tag="h", name="h")
                    nc.scalar.activation(
                        out=h[:], in_=ph[:], func=AF.Relu,
                        bias=bias_sb[:, colh + m:colh + m + 1],
                        scale=1.0 / SWG,
                    )
                    h_tiles.append(h)
                for m in range(MCH):
                    yap = ybf[:, m, base:base + SC]
                    d = dp.tile([128, SC], BF16, tag="d", name="d")
                    nc.vector.tensor_tensor(
                        out=d[:], in0=h_tiles[m][:], in1=yap, op=OP.subtract)
                    nc.vector.tensor_tensor(
                        out=d[:], in0=g_tiles[m][:], in1=d[:], op=OP.mult)
                    nc.vector.tensor_tensor(
                        out=yap, in0=yap, in1=d[:], op=OP.add)
                emit_pieces_sc(ybf, pieces_next, sc)
                if post_sc is not None:
                    post_sc(sc)

        # block 0
        p_y0p = alloc_pieces("y0p" in ACOMP, "y0p")
        hw_phase(0, 0, "h0l0", ybf0, yq0, None, wg0_sb, wh0_sb,
                 None, 1.0 / (SWG * S0), pieces_next=p_y0p)
        dump(1, ybf0)
        p_y0pp = alloc_pieces("y0pp" in ACOMP, "y0pp")
        hw_phase(0, 1, "h0l1", ybf0, p_y0p[0], p_y0p[1], wg0_sb, wh0_sb,
                 None, 1.0 / (SWG * S0), pieces_next=p_y0pp)
        dump(2, ybf0)
        yq0pp, ylo0pp = p_y0pp

        # ---- conv1 (fp8 DoubleRow; residual identity in center tap) ----
        ybf1 = ybfp.tile([128, KCH, T], BF16, tag="ybf", name="ybf1")
        p_y1 = alloc_pieces("y1" in ACOMP, "y1")
        for sc in range(NSC):
            base = sc * SC
            for m in range(MCH):
                dr_gemm.m = m
                pc = ps.tile([128, SC], F32, tag="ps", name="pc1")
                groups = [
                    (lambda j, m: c1hi_sb[:, 1, j, :, m * 128:(m + 1) * 128],
                     yq0pp, 0),
                    (lambda j, m: c1hi_sb[:, 0, j, :, m * 128:(m + 1) * 128],
                     yq0pp, -1),
                    (lambda j, m: c1hi_sb[:, 2, j, :, m * 128:(m + 1) * 128],
                     yq0pp, 1),
                ]
                for k, dt in ((1, 0), (0, -1), (2, 1)):
                    groups.append(
                        (lambda j, m, k=k: c1lo_sb[:, k, j, :,
                                                   m * 128:(m + 1) * 128],
                         yq0pp, dt))
                if ylo0pp is not None:
                    for k, dt in ((1, 0), (0, -1), (2, 1)):
                        groups.append(
                            (lambda j, m, k=k: c1hi_sb[:, k, j, :,
                                                       m * 128:(m + 1) * 128],
                             ylo0pp, dt))
                dr_gemm(pc, base, groups)
                nc.scalar.activation(
                    out=ybf1[:, m, base:base + SC], in_=pc[:],
                    func=AF.Relu, bias=bias_sb[:, 4 + m:5 + m],
                )
            emit_pieces_sc(ybf1, p_y1, sc)

        dump(3, ybf1)
        p_y1p = alloc_pieces("y1p" in ACOMP, "y1p")
        hw_phase(1, 0, "h1l0", ybf1, p_y1[0], p_y1[1], wg1_sb, wh1_sb,
                 wh1lo_sb if "h1l0" in WCOMP else None, 1.0 / (SWG * U1),
                 pieces_next=p_y1p)
        dump(4, ybf1)

        # ---- tail (emitted per-sc inside h1l1): transpose ybf1 ->
        # token-major y1t in DRAM; then per word-chunk gather + max tree +
        # proj as soon as the needed y1t rows exist ----
        a2_all = const.tile([128, KCH, W], BF16, name="a2_all")

        def seg_chunk(wc):
            tap = gat.tile([128, KCH, ntaps * 128], BF16, tag="tap", name="tap")
            rmax = min(((ntaps * 128 * (wc + 1) + 127) // 128) * 128, T)
            nc.gpsimd.dma_gather(
                out_ap=tap[:],
                in_ap=y1t_d[0:rmax, :],
                idxs_ap=gidx_sb[:, wc * ntaps * 8:(wc + 1) * ntaps * 8],
                num_idxs=ntaps * 128,
                num_idxs_reg=ntaps * 128,
                elem_size=DW,
                transpose=True,
                single_packet=False,
            )
            a2s = a2_all[:, :, wc * 128:(wc + 1) * 128]
            nc.vector.tensor_tensor(
                out=a2s, in0=tap[:, :, 0:128], in1=tap[:, :, 128:256], op=OP.max)
            for j in range(2, ntaps):
                nc.vector.tensor_tensor(
                    out=a2s, in0=a2s, in1=tap[:, :, j * 128:(j + 1) * 128],
                    op=OP.max)
            # proj (bf16; 1/U1 folded into projw) + bias, fp32 out
            po = tps.tile([128, DW], F32, tag="tps", name="po")
            for k in range(KCH):
                nc.tensor.matmul(
                    out=po[:],
                    lhsT=a2_all[:, k, wc * 128:(wc + 1) * 128],
                    rhs=projw_sb[:, k, :],
                    start=(k == 0),
                    stop=False,
                )
            nc.tensor.matmul(
                out=po[:], lhsT=ones_sb[:, 0:128], rhs=projb_sb[:],
                start=False, stop=True,
            )
            ob = obp.tile([128, DW], F32, tag="ob", name="ob")
            nc.scalar.activation(out=ob[:], in_=po[:], func=AF.Copy)
            nc.sync.dma_start(out=out_d[wc * 128:(wc + 1) * 128, :], in_=ob[:])

        def tail_sc(sc):
            for i in range(SC // 128):
                pt = tps.tile([128, 512], BF16, tag="tps", name="pt")
                for m in range(MCH):
                    nc.tensor.transpose(
                        out=pt[:, m * 128:(m + 1) * 128],
                        in_=ybf1[:, m, sc * SC + i * 128:sc * SC + (i + 1) * 128],
                        identity=ident_sb[:],
                    )
                st = tp.tile([128, 512], BF16, tag="y1t", name="st")
                nc.vector.tensor_copy(out=st[:], in_=pt[:])
                row0 = (sc * (SC // 128) + i) * 128
                nc.sync.dma_start(out=y1t_d[row0:row0 + 128, :], in_=st[:])
            hi = (sc + 1) * SC
            lo_r = sc * SC
            for wc in range(8):
                rmax = min(((ntaps * 128 * (wc + 1) + 127) // 128) * 128, T)
                ready_now = rmax <= hi
                ready_before = rmax <= lo_r
                if ready_now and not ready_before:
                    seg_chunk(wc)
                elif sc == NSC - 1 and not ready_now:
                    seg_chunk(wc)

        hw_phase(1, 1, "h1l1", ybf1, p_y1p[0], p_y1p[1], wg1_sb, wh1_sb,
                 wh1lo_sb if "h1l1" in WCOMP else None, 1.0 / (SWG * U1),
                 post_sc=tail_sc)
        dump(5, ybf1)

    nc.compile()
    return nc


@functools.lru_cache(maxsize=2)
def _program(ntaps: int) -> bass.Bass:
    return build_program(ntaps)


def _pack_idx(lin: np.ndarray) -> np.ndarray:
    """SWDGE idx layout: [128, N/16] int16, value n at [p, s] with
    n = s*16 + p%16, replicated across the eight 16-partition groups."""
    n = len(lin)
    assert n % 16 == 0
    arr = np.asarray(lin, dtype=np.int16).reshape(n // 16, 16).T
    return np.tile(arr, (8, 1)).copy()


def _q8(x):
    return np.clip(x, -240.0, 240.0).astype(f8_np)


def _wpack(w, sw, lo=False):
    """[512, 512] weight -> hi or lo fp8 piece, layout [128, KP, 2, 512]."""
    ws = w * sw
    hi = _q8(ws).astype(np.float32)
    src = (ws - hi) if lo else ws
    q = _q8(src)                                   # [512 in, 512 out]
    # in-feature f = (2j+i)*128 + p  ->  [p, j, i, out]
    return np.ascontiguousarray(
        np.transpose(q.reshape(KP, 2, 128, DW), (2, 0, 1, 3)))


def prepare(inputs):
    f32 = np.float32
    bt = np.asarray(inputs["byte_tokens"]).astype(np.int64)
    bpe = np.asarray(inputs["bpe_mask"]).astype(np.int64)
    wrd = np.asarray(inputs["word_mask"]).astype(np.int64)
    seg = np.asarray(inputs["seg_ids"]).astype(np.int64)
    emb = np.asarray(inputs["tok_emb"], dtype=f32)
    conv0_w = np.asarray(inputs["conv0_w"], dtype=f32)
    conv0_b = np.asarray(inputs["conv0_b"], dtype=f32)
    conv1_w = np.asarray(inputs["conv1_w"], dtype=f32)
    conv1_b = np.asarray(inputs["conv1_b"], dtype=f32)
    hw_w = {
        (0, "g"): np.asarray(inputs["hw0_wg"], dtype=f32),
        (0, "h"): np.asarray(inputs["hw0_wh"], dtype=f32),
        (1, "g"): np.asarray(inputs["hw1_wg"], dtype=f32),
        (1, "h"): np.asarray(inputs["hw1_wh"], dtype=f32),
    }
    hw_b = {
        (0, "g"): np.asarray(inputs["hw0_bg"], dtype=f32),
        (0, "h"): np.asarray(inputs["hw0_bh"], dtype=f32),
        (1, "g"): np.asarray(inputs["hw1_bg"], dtype=f32),
        (1, "h"): np.asarray(inputs["hw1_bh"], dtype=f32),
    }
    proj_w = np.asarray(inputs["proj_w"], dtype=f32)
    proj_b = np.asarray(inputs["proj_b"], dtype=f32)

    def as_bf16(x):
        return np.ascontiguousarray(x.astype(bf16_np))

    # combined embedding table (bf16)
    embc = np.zeros((CVOCAB, 128), f32)
    for bm in range(2):
        for wm in range(2):
            r0 = VOCAB * (bm + 2 * wm)
            embc[r0:r0 + VOCAB, :DB] = (
                emb + bm * emb[BPE_MARK] + wm * emb[WORD_MARK])

    def hwpack(block, gh, lo=False):
        # [NH, 512, 512] -> [128, NH, KP, 2, 512] fp8 at scale SWG
        return np.ascontiguousarray(np.stack(
            [_wpack(hw_w[(block, gh)][l], SWG, lo) for l in range(NH)],
            axis=1))

    c1 = conv1_w.copy()
    c1[1] += np.eye(DW, dtype=f32)

    def c1pack(lo=False):
        return np.ascontiguousarray(np.stack(
            [_wpack(c1[k], SW1, lo) for k in range(3)], axis=1))

    shared = {
        "emb_comb": as_bf16(embc),
        "w0": as_bf16(np.transpose(conv0_w * S0, (1, 0, 2))),    # [64,3,512]
        "wg0": hwpack(0, "g"),
        "wh0": hwpack(0, "h"),
        "wg1": hwpack(1, "g"),
        "wh1": hwpack(1, "h"),
        "wh1lo": hwpack(1, "h", lo=True),
        "c1hi": c1pack(),
        "c1lo": c1pack(lo=True),
        "projw": as_bf16(np.transpose(
            (proj_w / U1).reshape(KCH, 128, DW), (1, 0, 2))),
        "projb": as_bf16(proj_b.reshape(1, DW)),
        "ident": np.eye(128, dtype=bf16_np),
    }

    # biases (columns): conv0*S0, conv1*U1, sigmoid raw, relu-h*S_carrier
    bias_h = np.zeros((128, 40), f32)
    bias_h[:, 0:4] = (conv0_b * S0).reshape(KCH, 128).T
    bias_h[:, 4:8] = (conv1_b * U1).reshape(KCH, 128).T
    for blk, Sc in ((0, S0), (1, U1)):
        for l in range(NH):
            bias_h[:, _col_bg(blk, l):_col_bg(blk, l) + 4] = \
                hw_b[(blk, "g")][l].reshape(KCH, 128).T
            bias_h[:, _col_bh(blk, l):_col_bh(blk, l) + 4] = \
                (hw_b[(blk, "h")][l] * Sc).reshape(KCH, 128).T
    shared["biases"] = np.ascontiguousarray(bias_h)

    # per-core seg prep
    counts = np.zeros((B, W), np.int64)
    for b in range(B):
        counts[b] = np.bincount(seg[b], minlength=W)[:W]
    assert (counts >= 1).all(), "empty segments unsupported"
    ntaps = max(int(counts.max()), 2)
    starts = np.zeros((B, W), np.int64)
    starts[:, 1:] = np.cumsum(counts, axis=1)[:, :-1]
    ends = starts + counts - 1

    in_maps = []
    for b in range(B):
        cidx = bt[b] + VOCAB * (bpe[b] + 2 * wrd[b])
        gl = np.empty(8 * ntaps * 128, np.int64)
        for wc in range(8):
            nvec = np.arange(ntaps * 128)
            wv = wc * 128 + (nvec % 128)
            jv = nvec // 128
            gl[wc * ntaps * 128:(wc + 1) * ntaps * 128] = np.minimum(
                starts[b, wv] + jv, ends[b, wv]
            )
        m = dict(shared)
        m["tok_idx"] = _pack_idx(cidx)
        m["gidx"] = np.concatenate(
            [_pack_idx(gl[wc * ntaps * 128:(wc + 1) * ntaps * 128])
             for wc in range(8)], axis=1
        ).copy()
        in_maps.append(m)
    return ntaps, in_maps


def _run(inputs, trace=False, **kwargs):
    ntaps, in_maps = prepare(inputs)
    nc = _program(ntaps)
    res = run_bass_kernel_spmd(
        nc, in_maps, core_ids=list(range(NCORES)), trace=trace, **kwargs
    )
    out = np.stack([res.results[b]["out"] for b in range(B)], axis=0)
    return out.astype(np.float32), res


def kernel(**inputs) -> np.ndarray:
    out, _ = _run(inputs, trace=False)
    return out


def run_traced(inputs, **kwargs):
    return _run(inputs, trace=True, **kwargs)
